# revision 1
# baseline (speedup 1.0000x reference)
"""BandhaAttention Trainium2 kernel.

Sharding: 8 cores = 2 (batch) x 4 (head groups of 4 heads).
Per core: qkv projection for its 4 heads (q/k produced transposed, v natural),
gated q, causal attention via transposed scores (tk on partitions), exp on ACT,
AV with V-stationary matmuls (ones column -> softmax sums for free),
normalization via gpsimd partition_broadcast, out-projection row-sharded.
Host sums the 4 partial outputs per batch.
"""

import os
import sys

import numpy as np

for p in ("/opt/trn_rl_repo", "/opt/trn_rl_repo/concourse"):
    if p not in sys.path and os.path.isdir(p):
        sys.path.insert(0, p)

import ml_dtypes

import concourse.bacc as bacc
import concourse.mybir as mybir
from concourse.bass_utils import run_bass_kernel_spmd
from concourse.tile import TileContext

BF16 = mybir.dt.bfloat16
F32 = mybir.dt.float32
AF = mybir.ActivationFunctionType

T = 2048
D = 1024
HD = 64
NH_LOC = 4      # heads per core
DL = NH_LOC * HD  # 256 local qkv channels
KT = D // 128   # 8 contraction chunks
NQ = T // 512   # 4 tq chunks of 512
NTT = T // 128  # 16 tiles of 128

TALA = [5, 6, 7, 8]

LAST = None  # last BassKernelResults (for profiling from test.py)


def build_nc(reps=1):
    nc = bacc.Bacc("TRN2", target_bir_lowering=False)
    xt_d = nc.dram_tensor("xt", [D, T], BF16, kind="ExternalInput")
    wqk_d = nc.dram_tensor("wqk", [D, 2 * DL], BF16, kind="ExternalInput")
    wv_d = nc.dram_tensor("wv", [D, DL], BF16, kind="ExternalInput")
    wout_d = nc.dram_tensor("wout", [DL, D], BF16, kind="ExternalInput")
    gate_d = nc.dram_tensor("gate", [DL, T], BF16, kind="ExternalInput")
    tri_d = nc.dram_tensor("tri", [128, 128], BF16, kind="ExternalInput")
    out_d = nc.dram_tensor("out", [T, D], F32, kind="ExternalOutput")

    with TileContext(nc) as tc:
      for rep in range(reps):
        with (
            tc.tile_pool(name=f"pers{rep}", bufs=2) as pers,
            tc.tile_pool(name=f"pc1{rep}", bufs=1) as pc1,
            tc.tile_pool(name=f"pv{rep}", bufs=NTT) as pv,
        ):
            # ---- constants ----
            tri = pc1.tile([128, 128], BF16, tag="tri", name="tri")
            nc.sync.dma_start(tri, tri_d[:, :])
            wout_big = pers.tile([128, 2 * D], BF16, tag="wout",
                                 name="wout_big", bufs=1)
            nc.sync.dma_start(
                wout_big.rearrange("p (a c) -> p a c", c=D),
                wout_d[:, :].rearrange("(a p) c -> p a c", p=128))
            wout_sb = [wout_big[:, c * D:(c + 1) * D] for c in range(2)]

            # persistent products of phase 1
            qp_sb = [pers.tile([128, T], BF16, tag="qp", name="qp_sb")
                     for _ in range(2)]
            kp_sb = [pers.tile([128, T], BF16, tag="kp", name="kp_sb")
                     for _ in range(2)]
            v_all = [pv.tile([128, NH_LOC * (HD + 1)], BF16, tag="vall",
                             name="v_all") for _ in range(NTT)]
            aoT = [pers.tile([128, T], BF16, tag="aoT", name="aoT")
                   for _ in range(2)]

            # ---- phase 1 + 2 interleaved ----
            with (
                tc.tile_pool(name=f"pin{rep}", bufs=KT) as pin,
                tc.tile_pool(name=f"pexp{rep}", bufs=2) as pexp,
                tc.tile_pool(name=f"poex{rep}", bufs=3) as poex,
                tc.tile_pool(name=f"psm{rep}", bufs=2) as psm,
                tc.tile_pool(name=f"pstg{rep}", bufs=2) as pstg,
                tc.tile_pool(name=f"psq{rep}", bufs=2, space="PSUM") as psq,
                tc.tile_pool(name=f"pst{rep}", bufs=2, space="PSUM") as pst,
                tc.tile_pool(name=f"pav{rep}", bufs=2, space="PSUM") as pav,
            ):
                # consolidated strided loads: one DMA per tensor
                # (HWDGE splits a single InstDMACopy across all 16 SDMA slots)
                gate_big = pin.tile([128, 2 * T], BF16, tag="gate",
                                    name="gate_big", bufs=1)
                nc.sync.dma_start(
                    gate_big.rearrange("p (a t) -> p a t", t=T),
                    gate_d[:, :].rearrange("(a p) t -> p a t", p=128))
                gate_sb = [gate_big[:, c * T:(c + 1) * T] for c in range(2)]
                wqk_big = pin.tile([128, KT * 2 * DL], BF16, tag="wqk",
                                   name="wqk_big", bufs=1)
                nc.sync.dma_start(
                    wqk_big.rearrange("p (a c) -> p a c", c=2 * DL),
                    wqk_d[:, :].rearrange("(a p) c -> p a c", p=128))
                wqk_sb = [wqk_big[:, kc * 2 * DL:(kc + 1) * 2 * DL]
                          for kc in range(KT)]
                wv_big = pin.tile([128, KT * DL], BF16, tag="wv",
                                  name="wv_big", bufs=1)
                nc.sync.dma_start(
                    wv_big.rearrange("p (a c) -> p a c", c=DL),
                    wv_d[:, :].rearrange("(a p) c -> p a c", p=128))
                wv_sb = [wv_big[:, kc * DL:(kc + 1) * DL] for kc in range(KT)]
                xt_big = pin.tile([128, KT * T], BF16, tag="xt",
                                  name="xt_big", bufs=1)
                for qr in range(4):  # quarters for earlier first-matmul start
                    nc.sync.dma_start(
                        xt_big[:, qr * 2 * T:(qr + 1) * 2 * T].rearrange(
                            "p (a t) -> p a t", t=T),
                        xt_d[qr * 256:(qr + 1) * 256, :].rearrange(
                            "(a p) t -> p a t", p=128))
                xt_sb = [xt_big[:, kc * T:(kc + 1) * T] for kc in range(KT)]

                def do_qk(m, n0, n1):  # m-tile of qT/kT, tq chunks [n0,n1)
                    dst = qp_sb[m] if m < 2 else kp_sb[m - 2]
                    for n in range(n0, n1):
                        ps = psq.tile([128, 512], F32, tag="psq", name="ps_qk")
                        for kc in range(KT):
                            nc.tensor.matmul(
                                ps,
                                lhsT=wqk_sb[kc][:, m * 128:(m + 1) * 128],
                                rhs=xt_sb[kc][:, n * 512:(n + 1) * 512],
                                start=(kc == 0), stop=(kc == KT - 1),
                            )
                        if m < 2:  # gate the queries while evacuating
                            nc.vector.tensor_mul(
                                dst[:, n * 512:(n + 1) * 512], ps,
                                gate_sb[m][:, n * 512:(n + 1) * 512])
                        else:
                            nc.vector.tensor_copy(
                                dst[:, n * 512:(n + 1) * 512], ps)

                def do_v(t):  # v natural t-tile (128, 256) -> v_all
                    ps = psq.tile([128, DL], F32, tag="psq", name="ps_v")
                    for kc in range(KT):
                        nc.tensor.matmul(
                            ps,
                            lhsT=xt_sb[kc][:, t * 128:(t + 1) * 128],
                            rhs=wv_sb[kc],
                            start=(kc == 0), stop=(kc == KT - 1),
                        )
                    src = ps.rearrange("p (h c) -> p h c", c=HD)
                    dst = v_all[t].rearrange("p (h c) -> p h c", c=HD + 1)
                    nc.vector.tensor_copy(dst[:, :, 0:HD], src)
                    nc.vector.memset(dst[:, :, HD:HD + 1], 1.0)

                expt = {}  # (h, i) -> tile covering tq cols [128*i, T)

                def do_st_piece(p, i, c0):
                    w_i = T - 128 * i
                    if c0 == 0:
                        e0 = pexp.tile([128, w_i], BF16, tag=f"e{i}", name="e0")
                        e1 = pexp.tile([128, w_i], BF16, tag=f"e{i}", name="e1")
                        expt[(2 * p, i)] = e0
                        expt[(2 * p + 1, i)] = e1
                    w = min(1024, w_i - c0)
                    sts = []
                    for hh in range(2):
                        st = pst.tile([128, 1024], F32, tag="st", name="st_ps")
                        lo, hi = hh * 64, hh * 64 + 64
                        for nn in range(0, w, 512):
                            wn = min(512, w - nn)
                            a = 128 * i + c0 + nn
                            nc.tensor.matmul(
                                st[:, nn:nn + wn],
                                lhsT=kp_sb[p][lo:hi, i * 128:(i + 1) * 128],
                                rhs=qp_sb[p][lo:hi, a:a + wn],
                                start=True, stop=True,
                            )
                        sts.append(st)
                    for hh, st in enumerate(sts):
                        e = expt[(2 * p + hh, i)]
                        nc.scalar.activation(
                            e[:, c0:c0 + w], st[:, 0:w], AF.Exp, scale=0.125)
                    if c0 == 0:  # causal band mask on leading 128 cols
                        for hh in range(2):
                            e = expt[(2 * p + hh, i)]
                            nc.vector.tensor_mul(e[:, 0:128], e[:, 0:128], tri)

                av_tiles = {}

                def do_av_part(p, hh, j, i0, i1):
                    h = 2 * p + hh
                    last_i = 4 * j + 3
                    if i0 == 0:
                        # pair-1 final chunk: use the idle qkv psum banks so
                        # its early matmuls can run as in-loop filler without
                        # competing with the projection accumulators
                        pool, tg = (psq, "psq") if j == 3 else (pav, "av")
                        av_tiles[(p, hh)] = pool.tile([128, 512], F32,
                                                      tag=tg, name="av_ps")
                    av = av_tiles[(p, hh)]
                    for i in range(i0, i1):
                        off = 512 * j - 128 * i
                        r = max(0, -off)  # 128*(i%4) on diagonal tiles
                        nc.tensor.matmul(
                            av[0:HD + 1, r:512],
                            lhsT=v_all[i][:, hh * 65 + p * 130:
                                          hh * 65 + p * 130 + 65],
                            rhs=expt[(h, i)][:, off + r:off + 512],
                            start=(i == 0), stop=(i == last_i),
                        )
                    if i1 != last_i + 1:
                        return
                    rc = psm.tile([1, 512], F32, tag="rc", name="rc_sb")
                    nc.vector.reciprocal(rc, av[HD:HD + 1, :])
                    bc = psm.tile([64, 512], F32, tag="bc", name="bc_sb")
                    nc.gpsimd.partition_broadcast(bc, rc)
                    nc.vector.tensor_mul(
                        aoT[p][hh * 64:hh * 64 + 64, j * 512:(j + 1) * 512],
                        av[0:HD, :], bc)

                stg_tiles = {}

                def do_proj(t, n):
                    po = pav.tile([128, 512], F32, tag="av", name="po_ps")
                    for c in range(2):
                        nc.tensor.matmul(
                            po,
                            lhsT=aoT[c][:, t * 128:(t + 1) * 128],
                            rhs=wout_sb[c][:, n * 512:(n + 1) * 512],
                            start=(c == 0), stop=(c == 1),
                        )
                    if n == 0:
                        stg_tiles[t] = pstg.tile([128, D], F32, tag="stg",
                                                 name="stg_sb")
                    stg = stg_tiles[t]
                    nc.vector.tensor_copy(stg[:, n * 512:(n + 1) * 512], po)
                    if n == 1:
                        nc.sync.dma_start(out_d[t * 128:(t + 1) * 128, :], stg)

                def qk_unit(m, n):
                    return lambda: do_qk(m, n, n + 1)

                def v_unit(t):
                    return lambda: do_v(t)

                def st_units(p, j):
                    units = []
                    for i in range(4 * j, 4 * j + 4):
                        w_i = T - 128 * i
                        for c0 in range(0, w_i, 1024):
                            units.append(
                                (lambda p=p, i=i, c0=c0: do_st_piece(p, i, c0)))
                    return units

                def av_units(p, j):
                    units = []
                    last_i = 4 * j + 3
                    for hh in range(2):
                        for i0 in range(0, last_i + 1, 4):
                            i1 = min(i0 + 4, last_i + 1)
                            units.append(
                                (lambda p=p, hh=hh, j=j, i0=i0, i1=i1:
                                 do_av_part(p, hh, j, i0, i1)))
                    return units

                def proj_units(j):
                    return [(lambda t=t, n=n: do_proj(t, n))
                            for t in range(4 * j, 4 * j + 4) for n in range(2)]

                def interleave(primary, filler):
                    fi = 0
                    for k, pu in enumerate(primary):
                        pu()
                        target = ((k + 1) * len(filler)) // len(primary)
                        while fi < target:
                            filler[fi]()
                            fi += 1
                    while fi < len(filler):
                        filler[fi]()
                        fi += 1

                # pair-0 q projection + first k chunk
                do_qk(0, 0, 4)
                do_qk(2, 0, 1)
                # pair-0 attention; v / k-p0 / q-p1 / k-p1 as PE filler,
                # front-loaded so iteration 3's qkv psum banks are free for
                # the final AV chunk (early tail overlap + earlier exp-slot
                # release for pair 1)
                av03_last = []
                for j in range(NQ):
                    filler = []
                    if j < 3:
                        filler.append(qk_unit(2, j + 1))  # k-p0 chunk j+1
                    if j > 0:
                        filler += av_units(0, j - 1)
                    if j < 2:
                        filler += [v_unit(t) for t in range(8 * j, 8 * j + 8)]
                        filler += [qk_unit(1, n) for n in (2 * j, 2 * j + 1)]
                    elif j == 2:
                        filler += [qk_unit(3, n) for n in range(4)]
                    else:
                        av03 = av_units(0, 3)
                        filler += [u for idx, u in enumerate(av03)
                                   if idx % 4 != 3]
                        av03_last = [u for idx, u in enumerate(av03)
                                     if idx % 4 == 3]
                    interleave(st_units(0, j), filler)
                for u in av03_last:
                    u()
                # pair-1 attention with projection as filler
                av3 = av_units(1, 3)   # parts: h0 i0=0,4,8,12; h1 same
                av3_early = [u for idx, u in enumerate(av3) if idx % 4 != 3]
                av3_last = [u for idx, u in enumerate(av3) if idx % 4 == 3]
                for j in range(NQ):
                    filler = []
                    if j > 0:
                        filler += av_units(1, j - 1)
                        filler += proj_units(j - 1)
                    if j == 3:
                        filler += av3_early
                    interleave(st_units(1, j), filler)
                for u in av3_last:
                    u()
                for u in proj_units(3):
                    u()
    nc.compile()
    return nc


def _prep_inputs(x, w_qkv, w_out, bandha_gate):
    bf = ml_dtypes.bfloat16
    t = np.arange(T)
    gate_full = np.empty((16, T), np.float64)
    for h in range(16):
        cyc = TALA[h % len(TALA)]
        gate_full[h] = 1.0 / (1.0 + np.exp(-bandha_gate[h, t % cyc].astype(np.float64)))
    tri = (np.arange(128)[None, :] >= np.arange(128)[:, None]).astype(bf)

    in_maps = []
    for c in range(8):
        b, g = c // 4, c % 4
        xt = np.ascontiguousarray(x[b].T).astype(bf)
        wqk = np.concatenate(
            [w_qkv[:, g * DL:(g + 1) * DL],
             w_qkv[:, D + g * DL:D + (g + 1) * DL]], axis=1).astype(bf)
        wv = np.ascontiguousarray(w_qkv[:, 2 * D + g * DL:2 * D + (g + 1) * DL]).astype(bf)
        wout = np.ascontiguousarray(w_out[g * DL:(g + 1) * DL, :]).astype(bf)
        gb = np.repeat(gate_full[4 * g:4 * g + 4].astype(np.float32), HD, axis=0).astype(bf)
        in_maps.append({"xt": xt, "wqk": wqk, "wv": wv, "wout": wout,
                        "gate": np.ascontiguousarray(gb), "tri": tri})
    return in_maps


def kernel(**inputs):
    global LAST
    x = np.asarray(inputs["x"], np.float32)
    w_qkv = np.asarray(inputs["w_qkv"], np.float32)
    w_out = np.asarray(inputs["w_out"], np.float32)
    bandha_gate = np.asarray(inputs["bandha_gate"], np.float32)

    in_maps = _prep_inputs(x, w_qkv, w_out, bandha_gate)
    nc = build_nc()
    res = run_bass_kernel_spmd(
        nc, in_maps, core_ids=list(range(8)),
        trace=os.environ.get("BANDHA_TRACE") == "1",
    )
    LAST = res
    outs = [r["out"] for r in res.results]
    full = np.empty((2, T, D), np.float32)
    for b in range(2):
        full[b] = outs[4 * b] + outs[4 * b + 1] + outs[4 * b + 2] + outs[4 * b + 3]
    return full



# revision 42
# speedup vs baseline: 1.2799x; 1.2799x over previous
"""BandhaAttention Trainium2 kernel (v3 — natural-layout AV, combined proj).

Sharding: 8 cores = 2 (batch) x 4 (head groups of 4 heads).
Per core, heads are processed as 2 pairs of 2 heads. Scores are computed
transposed (keys on partitions), AV runs in natural layout: av[q, 65]
accumulates over key tiles with exp tiles as stationary weights,
streaming v plus a ones column (softmax sums for free). Normalization is
a per-partition scalar divide on DVE, the [q, c] -> [c, q] transpose for
the out-projection is done by the DMA xbar (SBUF->SBUF), and the
out-projection contracts both pairs into one bf16 partial output per
core (host sums 8 partials). Pair 0 walks stripes 0..3 while pair 1
walks 1,2,3,0 one super-stripe behind, so the exp workload on ACT
spreads evenly and the final stripe (pair 1, stripe 0) is tiny, keeping
the drain short. qkv projection for pair 1, v projection, and the
out-projections run as PE filler inside the attention stripes.
"""

import os
import sys

import numpy as np

for p in ("/opt/trn_rl_repo", "/opt/trn_rl_repo/concourse"):
    if p not in sys.path and os.path.isdir(p):
        sys.path.insert(0, p)

import ml_dtypes

import concourse.bacc as bacc
import concourse.mybir as mybir
from concourse.bass_utils import run_bass_kernel_spmd
from concourse.tile import TileContext

BF16 = mybir.dt.bfloat16
F32 = mybir.dt.float32
AF = mybir.ActivationFunctionType
ALU = mybir.AluOpType

T = 2048
D = 1024
HD = 64
KT = 8          # contraction chunks of 128 for the qkv projection

TALA = [5, 6, 7, 8]

LAST = None  # last BassKernelResults (for profiling from test.py)


def build_nc(reps=1):
    nc = bacc.Bacc("TRN2", target_bir_lowering=False)
    xt_d = nc.dram_tensor("xt", [D, T], BF16, kind="ExternalInput")
    wqk_d = nc.dram_tensor("wqk", [D, 512], BF16, kind="ExternalInput")
    wv_d = nc.dram_tensor("wv", [D, 256], BF16, kind="ExternalInput")
    wout_d = nc.dram_tensor("wout", [128, 2 * D], BF16, kind="ExternalInput")
    gate_d = nc.dram_tensor("gate", [256, T], BF16, kind="ExternalInput")
    tri_d = nc.dram_tensor("tri", [128, 128], BF16, kind="ExternalInput")
    eye_d = nc.dram_tensor("eye", [128, 128], BF16, kind="ExternalInput")
    out_d = nc.dram_tensor("out", [T, D], BF16, kind="ExternalOutput")

    with TileContext(nc) as tc:
      for rep in range(reps):
        with (
            tc.tile_pool(name=f"pers{rep}", bufs=1) as pers,
            tc.tile_pool(name=f"pexp{rep}", bufs=34) as pexp,
            tc.tile_pool(name=f"pao{rep}", bufs=32) as pao,
            tc.tile_pool(name=f"pstg{rep}", bufs=4) as pstg,
            tc.tile_pool(name=f"prc{rep}", bufs=4) as prc,
            tc.tile_pool(name=f"psq{rep}", bufs=2, space="PSUM") as psq,
            tc.tile_pool(name=f"pst{rep}", bufs=2, space="PSUM") as pst,
            tc.tile_pool(name=f"pav{rep}", bufs=2, space="PSUM") as pav,
        ):
            # ---- persistent SBUF tiles ----
            xt_sb = pers.tile([128, KT, T], BF16, tag="xt", name="xt_sb")
            wqk_sb = pers.tile([128, KT, 512], BF16, tag="wqk", name="wqk_sb")
            wv_sb = pers.tile([128, KT, 256], BF16, tag="wv", name="wv_sb")
            wout_sb = pers.tile([128, 2, D], BF16, tag="wout", name="wout_sb")
            gate_sb = pers.tile([128, 2, T], BF16, tag="gate", name="gate_sb")
            tri = pers.tile([128, 128], BF16, tag="tri", name="tri")
            eye = pers.tile([128, 128], BF16, tag="eye", name="eye")
            qp = [pers.tile([128, T], BF16, tag=f"qp{p}", name=f"qp{p}")
                  for p in range(2)]
            kp = [pers.tile([128, T], BF16, tag=f"kp{p}", name=f"kp{p}")
                  for p in range(2)]
            v8 = pers.tile([128, 16, 4, 65], BF16, tag="v8", name="v8")

            # ---- DMA preamble, ordered for earliest first matmul ----
            nc.sync.dma_start(
                wqk_sb[:, 0:2, :],
                wqk_d[0:256, :].rearrange("(a p) c -> p a c", p=128))
            for kc in range(2):
                nc.sync.dma_start(
                    xt_sb[:, kc, 0:512],
                    xt_d[kc * 128:(kc + 1) * 128, 0:512])
            nc.sync.dma_start(
                wqk_sb[:, 2:4, :],
                wqk_d[256:512, :].rearrange("(a p) c -> p a c", p=128))
            for kc in range(2, 4):
                nc.sync.dma_start(
                    xt_sb[:, kc, 0:512],
                    xt_d[kc * 128:(kc + 1) * 128, 0:512])
            nc.sync.dma_start(
                wqk_sb[:, 4:KT, :],
                wqk_d[512:D, :].rearrange("(a p) c -> p a c", p=128))
            for kc in range(4, 6):
                nc.sync.dma_start(
                    xt_sb[:, kc, 0:512],
                    xt_d[kc * 128:(kc + 1) * 128, 0:512])
            nc.sync.dma_start(gate_sb[:, 0, 0:512], gate_d[0:128, 0:512])
            for kc in range(6, KT):
                nc.sync.dma_start(
                    xt_sb[:, kc, 0:512],
                    xt_d[kc * 128:(kc + 1) * 128, 0:512])
            nc.sync.dma_start(tri, tri_d[:, :])
            nc.sync.dma_start(eye, eye_d[:, :])
            nc.sync.dma_start(
                wv_sb, wv_d[:, :].rearrange("(a p) c -> p a c", p=128))
            nc.sync.dma_start(gate_sb[:, 1, 0:512], gate_d[128:256, 0:512])
            nc.sync.dma_start(gate_sb[:, 0, 512:T], gate_d[0:128, 512:T])
            nc.sync.dma_start(gate_sb[:, 1, 512:T], gate_d[128:256, 512:T])
            for kc in range(KT):
                nc.sync.dma_start(
                    xt_sb[:, kc, 512:1024],
                    xt_d[kc * 128:(kc + 1) * 128, 512:1024])
            for kc in range(KT):
                nc.sync.dma_start(
                    xt_sb[:, kc, 1024:T],
                    xt_d[kc * 128:(kc + 1) * 128, 1024:T])
            nc.sync.dma_start(
                wout_sb.rearrange("p a c -> p (a c)"), wout_d[:, :])

            # v8 ones columns (softmax denominators via matmul)
            nc.gpsimd.memset(v8[:, :, :, 64:65], 1.0)

            # ---- engine work units ----

            def do_qk(m, n):
                """qkv projection m-tile (0:q-p0 1:q-p1 2:k-p0 3:k-p1),
                column chunk n (512 queries)."""
                ps = psq.tile([128, 512], F32, tag="psq", name="ps_qk")
                for kc in range(KT):
                    nc.tensor.matmul(
                        ps,
                        lhsT=wqk_sb[:, kc, m * 128:(m + 1) * 128],
                        rhs=xt_sb[:, kc, n * 512:(n + 1) * 512],
                        start=(kc == 0), stop=(kc == KT - 1),
                    )
                p = m % 2
                dst = (qp if m < 2 else kp)[p]
                if m < 2:  # gate the queries while evacuating
                    nc.vector.tensor_mul(
                        dst[:, n * 512:(n + 1) * 512], ps,
                        gate_sb[:, p, n * 512:(n + 1) * 512])
                else:
                    nc.vector.tensor_copy(dst[:, n * 512:(n + 1) * 512], ps)

            def do_v(t):
                """v natural projection for key tile t -> v8[:, t, :, 0:64]."""
                ps = psq.tile([128, 256], F32, tag="psq", name="ps_v")
                for kc in range(KT):
                    nc.tensor.matmul(
                        ps,
                        lhsT=xt_sb[:, kc, t * 128:(t + 1) * 128],
                        rhs=wv_sb[:, kc, :],
                        start=(kc == 0), stop=(kc == KT - 1),
                    )
                nc.vector.tensor_copy(
                    v8[:, t, :, 0:64],
                    ps.rearrange("p (h c) -> p h c", c=64))

            expt = {}  # (pair, i, j) -> [128, 2, 512] bf16 tile

            def do_st(p, i, j):
                """scores^T piece: key tile i, stripe j, both heads of pair
                p, exp'd into expt[(p, i, j)][:, :, r:512]."""
                r = max(0, 128 * i - 512 * j)
                w = 512 - r
                a = 512 * j + r
                st = pst.tile([128, 1024], F32, tag="st", name="st_ps")
                stv = st.rearrange("p (h c) -> p h c", c=512)
                for hh in range(2):
                    lo = hh * 64
                    nc.tensor.matmul(
                        stv[:, hh, r:512],
                        lhsT=kp[p][lo:lo + 64, i * 128:(i + 1) * 128],
                        rhs=qp[p][lo:lo + 64, a:a + w],
                        start=True, stop=True,
                    )
                e = pexp.tile([128, 2, 512], BF16, tag="exp", name="exp_sb")
                expt[(p, i, j)] = e
                nc.scalar.activation(
                    e[:, :, r:512], stv[:, :, r:512], AF.Exp, scale=0.125)
                if i >= 4 * j:  # diagonal piece: causal band mask on gpsimd
                    for hh in range(2):
                        nc.gpsimd.tensor_mul(
                            e[:, hh, r:r + 128], e[:, hh, r:r + 128], tri)

            aoN = {}

            def do_chain(p, hh, b):
                """natural AV for head hh of pair p, query block b: accumulate
                av[q, 65] over key tiles 0..b, then normalize into aoN."""
                j = b // 4
                av = pav.tile([128, 512], F32, tag="av", name="av_ps")
                for i in range(b + 1):
                    e = expt[(p, i, j)]
                    qq = 128 * (b - 4 * j)
                    nc.tensor.matmul(
                        av[:, 0:65],
                        lhsT=e[:, hh, qq:qq + 128],
                        rhs=v8[:, i, 2 * p + hh, :],
                        start=(i == 0), stop=(i == b),
                    )
                if (p, b) not in aoN:
                    aoN[(p, b)] = pao.tile([128, 128], BF16, tag="aoN",
                                           name="aoN_sb")
                rc = prc.tile([128, 1], F32, tag="rc", name="rc_sb")
                nc.vector.reciprocal(rc, av[:, 64:65])
                nc.vector.tensor_scalar(
                    aoN[(p, b)][:, hh * 64:hh * 64 + 64],
                    av[:, 0:64], rc, None, ALU.mult)

            aoT = {}

            def do_transpose(p, b, via="dma"):
                aoT[(p, b)] = pao.tile([128, 128], BF16, tag="aoT",
                                       name="aoT_sb")
                if via == "pe":
                    # tail blocks: PE transpose + ACT evac beats the ~2.3us
                    # DMA xbar latency; the pst ring is idle by then
                    tp = pst.tile([128, 128], BF16, tag="st", name="tp_ps")
                    nc.tensor.transpose(tp, aoN[(p, b)], eye)
                    nc.scalar.copy(aoT[(p, b)], tp)
                else:
                    nc.sync.dma_start(aoT[(p, b)], aoN[(p, b)], transpose=True)

            stg_tiles = {}

            def do_proj(t, n, evac="dve", split=False):
                """out-projection for query block t, column half n: contract
                both pairs (256 channels) into one po, evacuate bf16, DMA
                the finished output out."""
                po = psq.tile([128, 512], F32, tag="psq", name="po_ps")
                for p in range(2):
                    nc.tensor.matmul(
                        po,
                        lhsT=aoT[(p, t)],
                        rhs=wout_sb[:, p, n * 512:(n + 1) * 512],
                        start=(p == 0), stop=(p == 1),
                    )
                if n == 0:
                    stg_tiles[t] = pstg.tile([128, D], BF16, tag="stg",
                                             name="stg_sb")
                stg = stg_tiles[t]
                if evac == "act":  # tail blocks: ACT is done with exp there
                    nc.scalar.copy(stg[:, n * 512:(n + 1) * 512], po)
                else:
                    nc.vector.tensor_copy(stg[:, n * 512:(n + 1) * 512], po)
                if split:  # tail: per-half DMA so the drain overlaps
                    nc.sync.dma_start(
                        out_d[t * 128:(t + 1) * 128, n * 512:(n + 1) * 512],
                        stg[:, n * 512:(n + 1) * 512])
                elif n == 1:
                    nc.sync.dma_start(out_d[t * 128:(t + 1) * 128, :], stg)

            # ---- schedule ----

            def chain_unit(p, b, via="dma"):
                def u():
                    do_chain(p, 0, b)
                    do_chain(p, 1, b)
                    do_transpose(p, b, via)
                # chains stream v tiles 0..b as matmul rhs
                return (u, tuple(f"v{i}" for i in range(b + 1)))

            def stripe_units(p, j, i0=0, i1=None, chains=True, via="dma"):
                units = []
                for i in range(i0, 4 * j + 4 if i1 is None else i1):
                    units.append((lambda p=p, i=i, j=j: do_st(p, i, j),
                                  (f"m{2 + p}n{i // 4}", f"m{p}n{j}")))
                    if chains and i >= 4 * j:
                        units.append(chain_unit(p, i, via))
                return units

            def merge(u1, u2):
                out = []
                i1 = i2 = 0
                n1, n2 = len(u1), len(u2)
                while i1 < n1 or i2 < n2:
                    if i2 >= n2 or (i1 < n1 and i1 * n2 <= i2 * n1):
                        out.append(u1[i1])
                        i1 += 1
                    else:
                        out.append(u2[i2])
                        i2 += 1
                return out

            emitted = set()

            def interleave(primary, filler):
                """primary: [(unit, needs)]; filler: [(name, unit)].
                Emits fillers proportionally, but pulls a named filler
                forward whenever an upcoming primary depends on it, so
                every engine stream stays dependency-ordered (engines
                execute in order; a consumer emitted before its producer
                would deadlock real hardware)."""
                fi = 0

                def fire(idx):
                    name, u = filler[idx]
                    if name not in emitted:
                        emitted.add(name)
                        u()

                for k, (pu, needs) in enumerate(primary):
                    for nm in needs:
                        if nm in emitted:
                            continue
                        hit = [idx for idx, (n2, _) in enumerate(filler)
                               if n2 == nm]
                        assert hit, f"dependency {nm} not in filler list"
                        fire(hit[0])
                    pu()
                    target = ((k + 1) * len(filler)) // len(primary)
                    while fi < target:
                        fire(fi)
                        fi += 1
                while fi < len(filler):
                    fire(fi)
                    fi += 1

            def qk_u(m, n):
                return (f"m{m}n{n}", lambda: do_qk(m, n))

            def v_u(t):
                return (f"v{t}", lambda: do_v(t))

            def proj_u(t, n, evac="dve"):
                return (f"proj{t}_{n}", lambda: do_proj(t, n, evac))

            # preamble: pair-0 q/k first column chunks, kc-interleaved so
            # both accumulators advance as each xt/wqk chunk lands
            ps_a = psq.tile([128, 512], F32, tag="psq", name="ps_qk")
            ps_b = psq.tile([128, 512], F32, tag="psq", name="ps_qk")
            for kc in range(KT):
                for m, ps in ((0, ps_a), (2, ps_b)):
                    nc.tensor.matmul(
                        ps,
                        lhsT=wqk_sb[:, kc, m * 128:(m + 1) * 128],
                        rhs=xt_sb[:, kc, 0:512],
                        start=(kc == 0), stop=(kc == KT - 1),
                    )
            nc.vector.tensor_mul(qp[0][:, 0:512], ps_a, gate_sb[:, 0, 0:512])
            nc.vector.tensor_copy(kp[0][:, 0:512], ps_b)
            emitted.update({"m0n0", "m2n0"})

            # Engines execute their streams in order, so every unit is
            # emitted after everything it depends on: v tiles land before the
            # chains that read them, q/k chunks a super-stripe before their
            # stripes, proj after both pairs' transposes. Pair 1 runs one
            # super-stripe behind pair 0; its stripe-3 scores/exp are
            # pre-computed inside ss3 so the final chain phase (ss4) has no
            # exp dependency and the drain stays short.
            # ss0: p0 s0
            interleave(
                stripe_units(0, 0),
                [v_u(0), qk_u(3, 0), v_u(1), qk_u(2, 1), v_u(2),
                 qk_u(0, 1), v_u(3), qk_u(1, 0)],
            )
            # ss1: p0 s1 + p1 s0
            interleave(
                merge(stripe_units(0, 1), stripe_units(1, 0)),
                [qk_u(3, 1), v_u(4), qk_u(1, 1), v_u(5), qk_u(2, 2),
                 v_u(6), qk_u(0, 2), v_u(7)],
            )
            # ss2: p0 s2 + p1 s1 | combined proj t0-3 possible now
            interleave(
                merge(stripe_units(0, 2), stripe_units(1, 1)),
                [qk_u(3, 2), v_u(8), qk_u(1, 2), v_u(9), qk_u(2, 3),
                 v_u(10), qk_u(0, 3), v_u(11), qk_u(1, 3), qk_u(3, 3)],
            )
            # ss3: p0 s3 + p1 s2 + first part of p1 s3 scores | proj t0-7
            interleave(
                merge(merge(stripe_units(0, 3), stripe_units(1, 2)),
                      stripe_units(1, 3, i1=4)),
                [v_u(12), v_u(13), v_u(14), v_u(15)]
                + [proj_u(t, n) for t in range(0, 8) for n in range(2)],
            )
            # ss4: rest of p1 s3 + its chains (PE transpose) | proj t8-11
            interleave(
                stripe_units(1, 3, i0=4, via="pe"),
                [proj_u(t, n) for t in range(8, 12) for n in range(2)],
            )
            # tail: proj t12-15, evacuation split across DVE and ACT,
            # per-half output DMA
            for t in range(12, 16):
                for n in range(2):
                    do_proj(t, n, "act" if n else "dve", split=True)
    nc.compile()
    return nc


def _prep_inputs(x, w_qkv, w_out, bandha_gate):
    bf = ml_dtypes.bfloat16
    t = np.arange(T)
    gate_full = np.empty((16, T), np.float64)
    for h in range(16):
        cyc = TALA[h % len(TALA)]
        gate_full[h] = 1.0 / (
            1.0 + np.exp(-bandha_gate[h, t % cyc].astype(np.float64)))
    tri = (np.arange(128)[None, :] >= np.arange(128)[:, None]).astype(bf)
    eye = np.eye(128, dtype=np.float32).astype(bf)

    in_maps = []
    for c in range(8):
        b, g = c // 4, c % 4
        xt = np.ascontiguousarray(x[b].T).astype(bf)
        # m-tiles: q-pair0, q-pair1, k-pair0, k-pair1 (128 channels each)
        q0 = w_qkv[:, g * 256:g * 256 + 128]
        q1 = w_qkv[:, g * 256 + 128:g * 256 + 256]
        k0 = w_qkv[:, D + g * 256:D + g * 256 + 128]
        k1 = w_qkv[:, D + g * 256 + 128:D + g * 256 + 256]
        wqk = np.concatenate([q0, q1, k0, k1], axis=1).astype(bf)
        wv = np.ascontiguousarray(
            w_qkv[:, 2 * D + g * 256:2 * D + (g + 1) * 256]).astype(bf)
        # wout rows: within-pair channel, cols: (pair, d)
        wo = np.stack([w_out[g * 256:g * 256 + 128, :],
                       w_out[g * 256 + 128:g * 256 + 256, :]], axis=1)
        wo = np.ascontiguousarray(wo.reshape(128, 2 * D)).astype(bf)
        # gate rows: (pair, within-pair channel); pair p = heads {2p, 2p+1}
        gb = np.repeat(gate_full[4 * g:4 * g + 4].astype(np.float32),
                       HD, axis=0).astype(bf)
        in_maps.append({"xt": xt, "wqk": wqk, "wv": wv, "wout": wo,
                        "gate": np.ascontiguousarray(gb), "tri": tri,
                        "eye": eye})
    return in_maps


def kernel(**inputs):
    global LAST
    x = np.asarray(inputs["x"], np.float32)
    w_qkv = np.asarray(inputs["w_qkv"], np.float32)
    w_out = np.asarray(inputs["w_out"], np.float32)
    bandha_gate = np.asarray(inputs["bandha_gate"], np.float32)

    in_maps = _prep_inputs(x, w_qkv, w_out, bandha_gate)
    nc = build_nc()
    res = run_bass_kernel_spmd(
        nc, in_maps, core_ids=list(range(8)),
        trace=os.environ.get("BANDHA_TRACE") == "1",
    )
    LAST = res
    full = np.empty((2, T, D), np.float32)
    for b in range(2):
        acc = np.zeros((T, D), np.float32)
        for g in range(4):
            acc += np.asarray(res.results[4 * b + g]["out"],
                              dtype=np.float32)
        full[b] = acc
    return full
